# revision 2
# baseline (speedup 1.0000x reference)
"""7x7 grayscale dilation (flat SE, zero padding) on Trainium2, 8 NeuronCores.

fp16 end-to-end; host converts f32<->fp16 AND un-transposes the output.
Separable max filter: H-cascade -> PE transpose -> V-cascade, stored W-major.

Work distribution (per 3-image group, 6 max passes/image = 18 pass-images):
  DVE : H1 (511-wide), H3, V1 (reads PSUM directly), 1x V2, V3    ~14.5us
  Pool: H2, 2x V2                                                 ~14.6us
  ACT : boundary relu ops only (zero-padding edges), ~190ns each
  PE  : 16 transpose matmuls per image
The PSUM->SBUF copy of the old design is gone: V-pass-1 consumes the
transposed image straight out of PSUM; the 6-wide halo is reconstructed
from frozen zeros in the A buffer plus two relu'd edge columns on ACT.
"""
import numpy as np

_CACHE = {}

N_CORES = 8
IMGS = 12  # images per core: 4 batches x 3 channels
H = W = 512


def _build_nc(groups=(3, 3, 3, 3), vu_pool_imgs=2, head='img_thalf'):
    from contextlib import ExitStack
    from concourse import bacc, tile, mybir
    from concourse.masks import make_identity

    F16 = mybir.dt.float16
    MAX = mybir.AluOpType.max
    RELU = mybir.ActivationFunctionType.Relu
    groups = list(groups)
    NG = len(groups)
    starts = [sum(groups[:g]) for g in range(NG)]
    assert sum(groups) == IMGS
    GMAX = max(groups)

    nc = bacc.Bacc("TRN2", target_bir_lowering=False)
    x_in = nc.dram_tensor("x", [IMGS, H, W], F16, kind="ExternalInput")
    # y[i] is W-major: y[i][c, r] = dilate(x)[r, c]; host un-transposes
    y_out = nc.dram_tensor("y", [IMGS, W, H], F16, kind="ExternalOutput")

    with tile.TileContext(nc) as tc:
        with ExitStack() as ctx:
            pool = ctx.enter_context(tc.tile_pool(name="p", bufs=1))
            psum1 = ctx.enter_context(
                tc.tile_pool(name="ps1", bufs=3, space="PSUM"))

            ident = pool.tile([128, 128], F16)
            make_identity(nc, ident[:])

            def halo_tile(tag, g):
                t = pool.tile([128, 4, g, 518], F16, tag=tag)
                nc.gpsimd.memset(t[:, :, :, 0:3], 0.0)
                nc.gpsimd.memset(t[:, :, :, 515:518], 0.0)
                return t

            # X: one slot per group (load dst / H-result in place)
            xs = [halo_tile(f"x{g}", groups[g]) for g in range(NG)]
            # A (pass-1 out), U (pass-2 out), D (pass-3 out): 2 rotating slots.
            # A positions {0,1,515,516} are frozen zeros: pass-1 main writes
            # only [3,514), the edge relu writes {2,514}; pass-2 reads the
            # zeros to reproduce the halo semantics.
            as_, us, ds = [], [], []
            for s in range(2):
                a_t = pool.tile([128, 4, GMAX, 518], F16, tag=f"a{s}")
                nc.gpsimd.memset(a_t[:, :, :, 0:2], 0.0)
                nc.gpsimd.memset(a_t[:, :, :, 515:517], 0.0)
                u_t = pool.tile([128, 4, GMAX, 518], F16, tag=f"u{s}")
                d_t = pool.tile([128, 4, GMAX, 512], F16, tag=f"d{s}")
                as_.append(a_t)
                us.append(u_t)
                ds.append(d_t)

            def emit_loads(g, headsplit=False):
                X = xs[g]
                for li in range(groups[g]):
                    i = starts[g] + li
                    src = x_in[i].rearrange("(t p) w -> p t w", p=128, t=4)
                    if headsplit and li == 0:
                        for t in range(4):
                            nc.sync.dma_start(
                                out=X[:, t: t + 1, li, 3:515],
                                in_=src[:, t: t + 1, :],
                            )
                    else:
                        nc.sync.dma_start(out=X[:, :, li, 3:515], in_=src)

            def h_casc(g, c0, c1, t0=0, t1=4):
                """H cascade for group g, image range [c0,c1), t range."""
                X, A, U = xs[g], as_[g % 2], us[g % 2]
                # pass 1 main: A[3:514] = max(X[3:514], X[4:515])
                nc.vector.tensor_tensor(
                    A[:, t0:t1, c0:c1, 3:514], X[:, t0:t1, c0:c1, 3:514],
                    X[:, t0:t1, c0:c1, 4:515], op=MAX)
                # pass 1 edges on ACT: A[{2,514}] = relu(X[{3,514}])
                nc.scalar.activation(
                    A[:, t0:t1, c0:c1, 2:515:512],
                    X[:, t0:t1, c0:c1, 3:515:511], RELU)
                # pass 2 on Pool: U[0:515] = max(A[0:515], A[2:517])
                nc.gpsimd.tensor_tensor(
                    U[:, t0:t1, c0:c1, 0:515], A[:, t0:t1, c0:c1, 0:515],
                    A[:, t0:t1, c0:c1, 2:517], op=MAX)
                # pass 3: X[3:515] = max(U[0:512], U[3:515])  (in place)
                nc.vector.tensor_tensor(
                    X[:, t0:t1, c0:c1, 3:515], U[:, t0:t1, c0:c1, 0:512],
                    U[:, t0:t1, c0:c1, 3:515], op=MAX)

            def transpose_img(g, li):
                """PE transpose of H-result image li into a PSUM tile, then
                V-pass-1 (DVE) + edge relus (ACT) straight out of PSUM."""
                X, A = xs[g], as_[g % 2]
                P = psum1.tile([128, 2048], F16, tag="P1")
                for j in range(4):
                    for t in range(4):
                        nc.tensor.matmul(
                            P[:, 512 * j + 128 * t: 512 * j + 128 * t + 128],
                            X[:, t, li, 3 + 128 * j: 3 + 128 * j + 128],
                            ident[:],
                            is_transpose=True,
                        )
                Pv = P[:].rearrange("p (j r) -> p j r", j=4, r=512)
                # V pass 1 main: A[3:514] = max(P[0:511], P[1:512])
                nc.vector.tensor_tensor(
                    A[:, :, li, 3:514], Pv[:, :, 0:511], Pv[:, :, 1:512],
                    op=MAX)
                # V pass 1 edges: A[{2,514}] = relu(P[{0,511}])
                nc.scalar.activation(
                    A[:, :, li, 2:515:512], Pv[:, :, 0:512:511], RELU)

            def v_tail(g):
                """V passes 2,3 + stores for group g."""
                A, U, D = as_[g % 2], us[g % 2], ds[g % 2]
                gsz = groups[g]
                np_ = min(vu_pool_imgs, gsz)
                # pass 2: U[0:515] = max(A[0:515], A[2:517])
                if np_ > 0:
                    nc.gpsimd.tensor_tensor(
                        U[:, :, 0:np_, 0:515], A[:, :, 0:np_, 0:515],
                        A[:, :, 0:np_, 2:517], op=MAX)
                if np_ < gsz:
                    nc.vector.tensor_tensor(
                        U[:, :, np_:gsz, 0:515], A[:, :, np_:gsz, 0:515],
                        A[:, :, np_:gsz, 2:517], op=MAX)
                # pass 3: D[0:512] = max(U[0:512], U[3:515])
                nc.vector.tensor_tensor(
                    D[:, :, 0:gsz, 0:512], U[:, :, 0:gsz, 0:512],
                    U[:, :, 0:gsz, 3:515], op=MAX)
                for li in range(gsz):
                    i = starts[g] + li
                    dst = y_out[i].rearrange("(c p) r -> p c r", p=128, c=4)
                    nc.scalar.dma_start(out=dst[:], in_=D[:, :, li, :])

            emit_loads(0, headsplit=True)
            for g in range(NG):
                if g + 1 < NG:
                    emit_loads(g + 1)
                if g == 0 and head == 'img_thalf':
                    # first image in t-pieces so compute starts as soon as
                    # the first quarter-image load lands
                    h_casc(g, 0, 1, 0, 1)
                    h_casc(g, 0, 1, 1, 2)
                    h_casc(g, 0, 1, 2, 4)
                    h_casc(g, 1, groups[g])
                else:
                    h_casc(g, 0, groups[g])
                for li in range(groups[g]):
                    transpose_img(g, li)
                if g >= 1:
                    v_tail(g - 1)
            v_tail(NG - 1)

    nc.finalize()
    return nc


def _get_nc():
    if "nc" not in _CACHE:
        _CACHE["nc"] = _build_nc()
    return _CACHE["nc"]


def _run_bass(x, trace=False):
    """x: (32,3,512,512) float32 -> (32,3,512,512) float32 via 8 cores."""
    import time
    from concourse.bass_utils import run_bass_kernel_spmd

    nc = _get_nc()
    xr = np.ascontiguousarray(x).astype(np.float16).reshape(N_CORES, IMGS, H, W)
    in_maps = [{"x": xr[k]} for k in range(N_CORES)]
    # retry transient device errors (e.g. NRT_EXEC_UNIT_UNRECOVERABLE hiccups)
    for attempt in range(3):
        try:
            r = run_bass_kernel_spmd(nc, in_maps, list(range(N_CORES)), trace=trace)
            break
        except Exception:
            if attempt == 2:
                raise
            time.sleep(15)
    out = np.stack([r.results[k]["y"] for k in range(N_CORES)], axis=0)
    # y is W-major per image: out[k, i, c, r] -> result[k, i, r, c]
    out = out.transpose(0, 1, 3, 2)
    return np.ascontiguousarray(out).reshape(32, 3, 512, 512).astype(np.float32), r


def kernel(x, se):
    x = np.asarray(x, dtype=np.float32)
    se = np.asarray(se, dtype=np.float32)
    if se.shape == (7, 7) and np.all(se == 1.0):
        out, _ = _run_bass(x)
        return out
    # general fallback (never hit for this problem's inputs)
    kh, kw = se.shape
    ph, pw = kh // 2, kw // 2
    bias = se.reshape(-1) - 1.0
    mask = (bias >= 0).astype(x.dtype)
    xp = np.pad(x, ((0, 0), (0, 0), (ph, ph), (pw, pw)))
    out = np.full(x.shape, -np.inf, dtype=x.dtype)
    for i in range(kh * kw):
        r, c = i // kw, i % kw
        win = xp[:, :, r: r + x.shape[2], c: c + x.shape[3]]
        out = np.maximum(out, mask[i] * win + bias[i])
    return out


# revision 5
# speedup vs baseline: 1.1412x; 1.1412x over previous
"""7x7 grayscale dilation (flat SE, zero padding) on Trainium2, 8 NeuronCores.

fp16 end-to-end; host converts f32<->fp16 AND un-transposes the output.
Separable max filter, 6 combining passes per image (information-theoretic
minimum):  H1,H2,H3 (row direction) -> PE transpose -> V1,V2 (col direction)
-> V3 fused into the stores (store U[0:512], then a second gpsimd DMA with
accum_op=max storing U[3:515] onto the same HBM region).

Work distribution (per-image ownership; no cross-engine ping-pong inside a
cascade):
  DVE : 9 images - H1,H2,H3, V1 (direct from PSUM), V2          ~50.6us
  Pool: 3 images - full cascade from SBUF copies + store2 DGE    ~51us
  ACT : PSUM->SBUF copies for Pool images + V-edge relus          ~9us
  PE  : 16 transpose matmuls per image
  DMA : loads + double stores                                    ~52.6us
V1 for DVE images reads the transposed image straight out of PSUM; the
6-wide zero halo is rebuilt from H1's zero-propagating writes plus two
relu'd edge columns on ACT.

SBUF tiles are laid out [128, img, t, 518] (image dim OUTER) so the grouped
accum-store can merge (img, t) into one AP dim.
"""
import numpy as np

_CACHE = {}

N_CORES = 8
IMGS = 12  # images per core: 4 batches x 3 channels
H = W = 512


def _build_nc(groups=(3, 3, 3, 3), pool_imgs=((0, 2), (1, 2), (2, 2))):
    from contextlib import ExitStack
    from concourse import bacc, tile, mybir
    from concourse.masks import make_identity

    F16 = mybir.dt.float16
    MAX = mybir.AluOpType.max
    RELU = mybir.ActivationFunctionType.Relu
    groups = list(groups)
    NG = len(groups)
    starts = [sum(groups[:g]) for g in range(NG)]
    assert sum(groups) == IMGS
    GMAX = max(groups)
    pool_set = set(pool_imgs)  # (g, li) pairs owned by the Pool engine

    def is_pool(g, li):
        return (g, li) in pool_set

    nc = bacc.Bacc("TRN2", target_bir_lowering=False)
    x_in = nc.dram_tensor("x", [IMGS, H, W], F16, kind="ExternalInput")
    # y[i] is W-major: y[i][c, r] = dilate(x)[r, c]; host un-transposes
    y_out = nc.dram_tensor("y", [IMGS, W, H], F16, kind="ExternalOutput")

    with tile.TileContext(nc) as tc:
        with ExitStack() as ctx:
            pool = ctx.enter_context(tc.tile_pool(name="p", bufs=1))
            psum1 = ctx.enter_context(
                tc.tile_pool(name="ps1", bufs=3, space="PSUM"))

            ident = pool.tile([128, 128], F16)
            make_identity(nc, ident[:])

            def halo_tile(tag, g):
                t = pool.tile([128, g, 4, 518], F16, tag=tag)
                nc.gpsimd.memset(t[:, :, :, 0:3], 0.0)
                nc.gpsimd.memset(t[:, :, :, 515:518], 0.0)
                return t

            # X: one slot per group (load dst / H-result in place)
            xs = [halo_tile(f"x{g}", groups[g]) for g in range(NG)]
            # A (pass-1 out), U (pass-2 out): 3 rotating slots (indexed g%3)
            # so WAR hazards against in-flight stores have 2 groups of slack.
            as_, us = [], []
            for s in range(3):
                a_t = pool.tile([128, GMAX, 4, 518], F16, tag=f"a{s}")
                u_t = pool.tile([128, GMAX, 4, 518], F16, tag=f"u{s}")
                as_.append(a_t)
                us.append(u_t)
            # VT: halo'd transposed-image tiles for Pool-owned images
            # (ACT copies PSUM into them); 2 rotating slots.
            vts = []
            for s in range(2):
                vt_t = pool.tile([128, 4, 518], F16, tag=f"vt{s}")
                nc.gpsimd.memset(vt_t[:, :, 0:3], 0.0)
                nc.gpsimd.memset(vt_t[:, :, 515:518], 0.0)
                vts.append(vt_t)
            n_vt = 0

            def emit_loads(g, headsplit=False):
                X = xs[g]
                for li in range(groups[g]):
                    i = starts[g] + li
                    src = x_in[i].rearrange("(t p) w -> p t w", p=128, t=4)
                    if headsplit and li == 0:
                        for t in range(4):
                            nc.sync.dma_start(
                                out=X[:, li, t: t + 1, 3:515],
                                in_=src[:, t: t + 1, :],
                            )
                    else:
                        nc.sync.dma_start(out=X[:, li, :, 3:515], in_=src)

            def h_img(g, c0, c1, eng):
                """H cascade passes 1-3 for images [c0,c1) of group g.
                Full-width ops: X's zero halo propagates so that A keeps
                zeros at {0,1,515,516} - V1-from-PSUM relies on this."""
                X, A, U = xs[g], as_[g % 3], us[g % 3]
                eng.tensor_tensor(
                    A[:, c0:c1, :, 0:517], X[:, c0:c1, :, 0:517],
                    X[:, c0:c1, :, 1:518], op=MAX)
                eng.tensor_tensor(
                    U[:, c0:c1, :, 0:515], A[:, c0:c1, :, 0:515],
                    A[:, c0:c1, :, 2:517], op=MAX)
                eng.tensor_tensor(
                    X[:, c0:c1, :, 3:515], U[:, c0:c1, :, 0:512],
                    U[:, c0:c1, :, 3:515], op=MAX)

            def h_phase(g):
                gsz = groups[g]
                ndve = gsz - sum(1 for li in range(gsz) if is_pool(g, li))
                if g == 0:
                    # per-image ops so compute starts right after each load
                    for li in range(ndve):
                        h_img(g, li, li + 1, nc.vector)
                elif ndve > 0:
                    h_img(g, 0, ndve, nc.vector)
                for li in range(ndve, gsz):
                    h_img(g, li, li + 1, nc.gpsimd)

            vt_of = {}

            def transposes(g):
                nonlocal n_vt
                X = xs[g]
                for li in range(groups[g]):
                    P = psum1.tile([128, 2048], F16, tag="P1")
                    for j in range(4):
                        for t in range(4):
                            nc.tensor.matmul(
                                P[:, 512 * j + 128 * t: 512 * j + 128 * t + 128],
                                X[:, li, t, 3 + 128 * j: 3 + 128 * j + 128],
                                ident[:],
                                is_transpose=True,
                            )
                    Pv = P[:].rearrange("p (j r) -> p j r", j=4, r=512)
                    if is_pool(g, li):
                        # ACT drains PSUM into a halo'd SBUF tile; the Pool
                        # engine runs V1/V2 from there at full width.
                        VT = vts[n_vt % 2]
                        vt_of[(g, li)] = VT
                        n_vt += 1
                        nc.scalar.copy(VT[:, :, 3:515], Pv)
                    else:
                        A = as_[g % 3]
                        # V1 main: A[3:514] = max(P[0:511], P[1:512])  (DVE)
                        nc.vector.tensor_tensor(
                            A[:, li, :, 3:514], Pv[:, :, 0:511],
                            Pv[:, :, 1:512], op=MAX)
                        # V1 edges: A[{2,514}] = relu(P[{0,511}])  (ACT)
                        nc.scalar.activation(
                            A[:, li, :, 2:515:512], Pv[:, :, 0:512:511], RELU)

            def v_phase(g):
                """V pass 2 (pass 1 for DVE images already ran with the
                transposes); writes U, which the store pair consumes."""
                A, U = as_[g % 3], us[g % 3]
                for li in range(groups[g]):
                    if is_pool(g, li):
                        VT = vt_of[(g, li)]
                        nc.gpsimd.tensor_tensor(
                            A[:, li, :, 0:517], VT[:, :, 0:517],
                            VT[:, :, 1:518], op=MAX)
                        nc.gpsimd.tensor_tensor(
                            U[:, li, :, 0:515], A[:, li, :, 0:515],
                            A[:, li, :, 2:517], op=MAX)
                    else:
                        nc.vector.tensor_tensor(
                            U[:, li, :, 0:515], A[:, li, :, 0:515],
                            A[:, li, :, 2:517], op=MAX)

            def store1s(g):
                U = us[g % 3]
                for li in range(groups[g]):
                    i = starts[g] + li
                    dst = y_out[i].rearrange("(c p) r -> p c r", p=128, c=4)
                    nc.sync.dma_start(out=dst[:], in_=U[:, li, :, 0:512])

            def store2(g):
                """V pass 3 happens inside the DMA engines: max-accumulate
                the 3-shifted window onto the already-stored y rows."""
                U = us[g % 3]
                gsz = groups[g]
                i0 = starts[g]
                dst = y_out[i0:i0 + gsz].rearrange(
                    "g (c p) r -> p (g c) r", p=128, c=4)
                src = U[:, 0:gsz, :, 3:515].rearrange("p g t r -> p (g t) r")
                nc.gpsimd.dma_start(out=dst[:], in_=src, accum_op=MAX)

            emit_loads(0, headsplit=True)
            for g in range(NG):
                if g + 1 < NG:
                    emit_loads(g + 1)
                h_phase(g)
                if g >= 1:
                    v_phase(g - 1)
                transposes(g)
                if g >= 1:
                    store1s(g - 1)
                if g >= 2:
                    store2(g - 2)
            v_phase(NG - 1)
            store1s(NG - 1)
            store2(NG - 2)
            store2(NG - 1)

    nc.finalize()
    return nc


def _get_nc():
    if "nc" not in _CACHE:
        _CACHE["nc"] = _build_nc()
    return _CACHE["nc"]


def _run_bass(x, trace=False):
    """x: (32,3,512,512) float32 -> (32,3,512,512) float32 via 8 cores."""
    import time
    from concourse.bass_utils import run_bass_kernel_spmd

    nc = _get_nc()
    xr = np.ascontiguousarray(x).astype(np.float16).reshape(N_CORES, IMGS, H, W)
    in_maps = [{"x": xr[k]} for k in range(N_CORES)]
    # retry transient device errors (e.g. NRT_EXEC_UNIT_UNRECOVERABLE hiccups)
    for attempt in range(3):
        try:
            r = run_bass_kernel_spmd(nc, in_maps, list(range(N_CORES)), trace=trace)
            break
        except Exception:
            if attempt == 2:
                raise
            time.sleep(15)
    out = np.stack([r.results[k]["y"] for k in range(N_CORES)], axis=0)
    # y is W-major per image: out[k, i, c, r] -> result[k, i, r, c]
    out = out.transpose(0, 1, 3, 2)
    return np.ascontiguousarray(out).reshape(32, 3, 512, 512).astype(np.float32), r


def kernel(x, se):
    x = np.asarray(x, dtype=np.float32)
    se = np.asarray(se, dtype=np.float32)
    if se.shape == (7, 7) and np.all(se == 1.0):
        out, _ = _run_bass(x)
        return out
    # general fallback (never hit for this problem's inputs)
    kh, kw = se.shape
    ph, pw = kh // 2, kw // 2
    bias = se.reshape(-1) - 1.0
    mask = (bias >= 0).astype(x.dtype)
    xp = np.pad(x, ((0, 0), (0, 0), (ph, ph), (pw, pw)))
    out = np.full(x.shape, -np.inf, dtype=x.dtype)
    for i in range(kh * kw):
        r, c = i // kw, i % kw
        win = xp[:, :, r: r + x.shape[2], c: c + x.shape[3]]
        out = np.maximum(out, mask[i] * win + bias[i])
    return out


# revision 6
# speedup vs baseline: 1.2366x; 1.0836x over previous
"""7x7 grayscale dilation (flat SE, zero padding) on Trainium2, 8 NeuronCores.

fp16 end-to-end; host converts f32<->fp16 AND un-transposes the output.
Separable max filter, 6 combining passes per image (information-theoretic
minimum):  H1,H2,H3 (row direction) -> PE transpose -> V1,V2 (col direction)
-> V3 fused into the stores (store U[0:512], then a second gpsimd DMA with
accum_op=max storing U[3:515] onto the same HBM region).

Per-image engine assignment (tunable via `assign`): most images run fully
on DVE; the Pool engine owns a couple of images end-to-end (via an ACT
PSUM->SBUF copy for its V phase) plus some H-phases, balancing
DVE ~ Pool ~ DMA ~ 52us of work each.  A/U buffers are per-group (no slot
reuse -> no WAR stalls against in-flight stores).  store2 is split so that
fast DVE images' stores never wait on a slower Pool-owned sibling, and the
last group's stores are emitted per image right after each V2.
"""
import numpy as np

_CACHE = {}

N_CORES = 8
IMGS = 12  # images per core: 4 batches x 3 channels
H = W = 512


def _default_assign():
    # per (g, li): h/v1/v2 owner: 'd'=DVE, 'p'=Pool; h may be 's' (t-split:
    # Pool does t 0:2, DVE t 2:4).  v1='p' implies ACT copies PSUM to SBUF.
    a = {(g, li): dict(h='d', v1='d', v2='d')
         for g in range(4) for li in range(3)}
    a[(0, 2)] = dict(h='p', v1='p', v2='p')
    a[(1, 2)] = dict(h='p', v1='p', v2='p')
    a[(2, 2)] = dict(h='p', v1='d', v2='d')
    a[(3, 2)] = dict(h='s', v1='d', v2='d')
    return a


# store2 li-ranges per group: DVE siblings' stores flow without waiting on
# the Pool-owned laggard.
_DEFAULT_S2 = {0: [(0, 2), (2, 3)], 1: [(0, 2), (2, 3)],
               2: [(0, 3)], 3: [(0, 2), (2, 3)]}


def _build_nc(groups=(3, 3, 3, 3), assign=None, s2split=None):
    from contextlib import ExitStack
    from concourse import bacc, tile, mybir
    from concourse.masks import make_identity

    F16 = mybir.dt.float16
    MAX = mybir.AluOpType.max
    RELU = mybir.ActivationFunctionType.Relu
    groups = list(groups)
    NG = len(groups)
    starts = [sum(groups[:g]) for g in range(NG)]
    assert sum(groups) == IMGS
    assign = assign or _default_assign()
    s2split = s2split or _DEFAULT_S2

    nc = bacc.Bacc("TRN2", target_bir_lowering=False)
    x_in = nc.dram_tensor("x", [IMGS, H, W], F16, kind="ExternalInput")
    # y[i] is W-major: y[i][c, r] = dilate(x)[r, c]; host un-transposes
    y_out = nc.dram_tensor("y", [IMGS, W, H], F16, kind="ExternalOutput")

    with tile.TileContext(nc) as tc:
        with ExitStack() as ctx:
            pool = ctx.enter_context(tc.tile_pool(name="p", bufs=1))
            psum1 = ctx.enter_context(
                tc.tile_pool(name="ps1", bufs=3, space="PSUM"))

            ident = pool.tile([128, 128], F16)
            make_identity(nc, ident[:])

            def halo_tile(tag, g):
                t = pool.tile([128, g, 4, 518], F16, tag=tag)
                nc.gpsimd.memset(t[:, :, :, 0:3], 0.0)
                nc.gpsimd.memset(t[:, :, :, 515:518], 0.0)
                return t

            # X / A / U: one dedicated slot per group (no reuse, no WAR)
            xs = [halo_tile(f"x{g}", groups[g]) for g in range(NG)]
            as_, us = [], []
            for g in range(NG):
                a_t = pool.tile([128, groups[g], 4, 518], F16, tag=f"a{g}")
                u_t = pool.tile([128, groups[g], 4, 518], F16, tag=f"u{g}")
                as_.append(a_t)
                us.append(u_t)
            # VT: halo'd transposed-image tiles for Pool-V images
            vts = []
            for s in range(2):
                vt_t = pool.tile([128, 4, 518], F16, tag=f"vt{s}")
                nc.gpsimd.memset(vt_t[:, :, 0:3], 0.0)
                nc.gpsimd.memset(vt_t[:, :, 515:518], 0.0)
                vts.append(vt_t)
            n_vt = 0

            def emit_loads(g):
                X = xs[g]
                for li in range(groups[g]):
                    i = starts[g] + li
                    src = x_in[i].rearrange("(t p) w -> p t w", p=128, t=4)
                    nc.sync.dma_start(out=X[:, li, :, 3:515], in_=src)

            def h_img(g, li, eng, t0=0, t1=4):
                """H cascade passes 1-3 for image li of group g."""
                X, A, U = xs[g], as_[g], us[g]
                eng.tensor_tensor(
                    A[:, li, t0:t1, 0:517], X[:, li, t0:t1, 0:517],
                    X[:, li, t0:t1, 1:518], op=MAX)
                eng.tensor_tensor(
                    U[:, li, t0:t1, 0:515], A[:, li, t0:t1, 0:515],
                    A[:, li, t0:t1, 2:517], op=MAX)
                eng.tensor_tensor(
                    X[:, li, t0:t1, 3:515], U[:, li, t0:t1, 0:512],
                    U[:, li, t0:t1, 3:515], op=MAX)

            def h_phase(g):
                for li in range(groups[g]):
                    h = assign[(g, li)]['h']
                    if h == 'd':
                        if g == 0 and li == 0:
                            # t-halves so DVE starts on the first half-image
                            h_img(g, li, nc.vector, 0, 2)
                            h_img(g, li, nc.vector, 2, 4)
                        else:
                            h_img(g, li, nc.vector)
                    elif h == 'p':
                        h_img(g, li, nc.gpsimd)
                    else:  # 's': split between Pool and DVE
                        h_img(g, li, nc.gpsimd, 0, 2)
                        h_img(g, li, nc.vector, 2, 4)

            vt_of = {}

            def transpose_img(g, li):
                nonlocal n_vt
                X = xs[g]
                P = psum1.tile([128, 2048], F16, tag="P1")
                for j in range(4):
                    for t in range(4):
                        nc.tensor.matmul(
                            P[:, 512 * j + 128 * t: 512 * j + 128 * t + 128],
                            X[:, li, t, 3 + 128 * j: 3 + 128 * j + 128],
                            ident[:],
                            is_transpose=True,
                        )
                Pv = P[:].rearrange("p (j r) -> p j r", j=4, r=512)
                A = as_[g]
                if assign[(g, li)]['v1'] == 'p':
                    # ACT drains PSUM into a halo'd SBUF tile; Pool runs
                    # V1/V2 from there at full width.
                    VT = vts[n_vt % 2]
                    vt_of[(g, li)] = VT
                    n_vt += 1
                    nc.scalar.copy(VT[:, :, 3:515], Pv)
                else:
                    # V1 main: A[3:514] = max(P[0:511], P[1:512])  (DVE)
                    nc.vector.tensor_tensor(
                        A[:, li, :, 3:514], Pv[:, :, 0:511],
                        Pv[:, :, 1:512], op=MAX)
                    # V1 edges: A[{2,514}] = relu(P[{0,511}])  (ACT)
                    nc.scalar.activation(
                        A[:, li, :, 2:515:512], Pv[:, :, 0:512:511], RELU)

            def v2_img(g, li):
                A, U = as_[g], us[g]
                if assign[(g, li)]['v1'] == 'p':
                    VT = vt_of[(g, li)]
                    nc.gpsimd.tensor_tensor(
                        A[:, li, :, 0:517], VT[:, :, 0:517],
                        VT[:, :, 1:518], op=MAX)
                eng = nc.gpsimd if assign[(g, li)]['v2'] == 'p' else nc.vector
                eng.tensor_tensor(
                    U[:, li, :, 0:515], A[:, li, :, 0:515],
                    A[:, li, :, 2:517], op=MAX)

            def store1(g, li):
                U = us[g]
                i = starts[g] + li
                dst = y_out[i].rearrange("(c p) r -> p c r", p=128, c=4)
                nc.sync.dma_start(out=dst[:], in_=U[:, li, :, 0:512])

            def store2(g, l0, l1):
                """V pass 3 happens inside the DMA engines: max-accumulate
                the 3-shifted window onto the already-stored y rows."""
                U = us[g]
                i0 = starts[g] + l0
                dst = y_out[i0:i0 + (l1 - l0)].rearrange(
                    "g (c p) r -> p (g c) r", p=128, c=4)
                src = U[:, l0:l1, :, 3:515].rearrange("p g t r -> p (g t) r")
                nc.gpsimd.dma_start(out=dst[:], in_=src, accum_op=MAX)

            def v_phase(g, interleave_stores=False):
                for li in range(groups[g]):
                    v2_img(g, li)
                    if interleave_stores:
                        store1(g, li)
                        for (l0, l1) in s2split[g]:
                            if l1 == li + 1:
                                store2(g, l0, l1)

            emit_loads(0)
            for g in range(NG):
                if g + 1 < NG:
                    emit_loads(g + 1)
                h_phase(g)
                if g >= 1:
                    v_phase(g - 1)
                for li in range(groups[g]):
                    transpose_img(g, li)
                if g >= 1:
                    for li in range(groups[g - 1]):
                        store1(g - 1, li)
                    for (l0, l1) in s2split[g - 1]:
                        store2(g - 1, l0, l1)
            v_phase(NG - 1, interleave_stores=True)

    nc.finalize()
    return nc


def _get_nc():
    if "nc" not in _CACHE:
        _CACHE["nc"] = _build_nc()
    return _CACHE["nc"]


def _run_bass(x, trace=False):
    """x: (32,3,512,512) float32 -> (32,3,512,512) float32 via 8 cores."""
    import time
    from concourse.bass_utils import run_bass_kernel_spmd

    nc = _get_nc()
    xr = np.ascontiguousarray(x).astype(np.float16).reshape(N_CORES, IMGS, H, W)
    in_maps = [{"x": xr[k]} for k in range(N_CORES)]
    # retry transient device errors (e.g. NRT_EXEC_UNIT_UNRECOVERABLE hiccups)
    for attempt in range(3):
        try:
            r = run_bass_kernel_spmd(nc, in_maps, list(range(N_CORES)), trace=trace)
            break
        except Exception:
            if attempt == 2:
                raise
            time.sleep(15)
    out = np.stack([r.results[k]["y"] for k in range(N_CORES)], axis=0)
    # y is W-major per image: out[k, i, c, r] -> result[k, i, r, c]
    out = out.transpose(0, 1, 3, 2)
    return np.ascontiguousarray(out).reshape(32, 3, 512, 512).astype(np.float32), r


def kernel(x, se):
    x = np.asarray(x, dtype=np.float32)
    se = np.asarray(se, dtype=np.float32)
    if se.shape == (7, 7) and np.all(se == 1.0):
        out, _ = _run_bass(x)
        return out
    # general fallback (never hit for this problem's inputs)
    kh, kw = se.shape
    ph, pw = kh // 2, kw // 2
    bias = se.reshape(-1) - 1.0
    mask = (bias >= 0).astype(x.dtype)
    xp = np.pad(x, ((0, 0), (0, 0), (ph, ph), (pw, pw)))
    out = np.full(x.shape, -np.inf, dtype=x.dtype)
    for i in range(kh * kw):
        r, c = i // kw, i % kw
        win = xp[:, :, r: r + x.shape[2], c: c + x.shape[3]]
        out = np.maximum(out, mask[i] * win + bias[i])
    return out
